# revision 14
# baseline (speedup 1.0000x reference)
"""DSimilarity.gradgrad force-force covariance block on 8 Trainium2 cores.

out[m*3+a, n*3+b] = sum_{i,j} u1[i,a]*u2[j,b]*gg[i,j]*[i1[i]==m]*[i2[j]==n]
with gg[i,j] = (c - c^2 diff^2) * exp(-0.5 c diff^2), diff = d1[i]-d2[j], c=1/l^2.

Strategy: out = S1T.T @ gg @ S2 with sparse scatter matrices densified after
sorting pairs by atom index. Axis-2 (j) is sorted by i2 and sharded 1/8 per
core -> each core produces a contiguous strip of output columns (overlap-add
at boundary atoms on the host). Axis-1 (i) is sorted by i1, packed tight to a
multiple of 128; stage B runs per 42-atom row block over the i-chunks that
block's pairs touch (boundary chunks appear in two blocks with disjoint
nonzero rows). gg never touches HBM: ACT/DVE/GpSimd compute it in SBUF
super-chunks, PE consumes it as matmul weights. d1 is replicated across
partitions with a K=1 ones-matmul (also warms the PE during the DMA preamble).
"""

import math
import sys
import types

import numpy as np

NCORES = 8
ABLK = 42  # atoms per stage-B row block (126 rows)

TRACE = False  # test.py sets True to capture an NTFF profile
LAST_RESULTS = None  # BassKernelResults of the last run (for test.py)

_PROGRAM_CACHE = {}


def _install_ntff_hook():
    try:
        from antenv.axon_hooks import get_axon_ntff_profile_hook  # noqa: F401
        return
    except ImportError:
        pass
    try:
        from trn_agent_boot.trn_boot import _ntff_profile_via_ctypes
        import antenv
        hook = _ntff_profile_via_ctypes('/opt/axon/libaxon_pjrt.so')
        mod = types.ModuleType("antenv.axon_hooks")
        mod._hook = hook
        mod.get_axon_ntff_profile_hook = lambda: mod._hook
        mod.set_axon_ntff_profile_hook = lambda h: setattr(mod, "_hook", h)
        antenv.axon_hooks = mod
        sys.modules["antenv.axon_hooks"] = mod
    except Exception:
        pass


def _sc_slices(ipad):
    """Split [0, ipad) into super-chunks of ~1536 (multiples of 128)."""
    out = []
    a = 0
    while a < ipad:
        w = min(1536, ipad - a)
        out.append((a, a + w))
        a += w
    return out


def _build_program(IPAD, NJ2, W3, insts, NBTOT, sqrtc, lnc):
    """Compile the per-core Bass program (same program on all 8 cores).

    insts: tuple of (block, chunk) stage-B instances, block-major.
    Matmuls run in float32r (tf32-class) with the moving dim padded to >=256
    so the PE streams 1 column/cycle; elementwise gg stays exact fp32.
    """
    import concourse.bacc as bacc
    import concourse.tile as tile
    import concourse.mybir as mybir

    F32 = mybir.dt.float32
    F32R = mybir.dt.float32r
    Alu = mybir.AluOpType
    Act = mybir.ActivationFunctionType

    NIC = IPAD // 128
    NBC = len(insts)
    NB = NBTOT

    # pad the stage-A/B moving dim to a multiple of 256, chunks of <=512
    W3P = ((W3 + 255) // 256) * 256
    col_chunks = []
    c0 = 0
    while c0 < W3P:
        col_chunks.append((c0, min(512, W3P - c0)))
        c0 += 512

    nc = bacc.Bacc("TRN2", target_bir_lowering=False, debug=False)
    d1_h = nc.dram_tensor("d1p", [1, IPAD], F32, kind="ExternalInput")
    d2_h = nc.dram_tensor("d2col", [128, NJ2], F32, kind="ExternalInput")
    s2_h = nc.dram_tensor("s2", [128, NJ2 * W3P], F32R, kind="ExternalInput")
    s1_h = nc.dram_tensor("s1t", [128, NBC * 126], F32R, kind="ExternalInput")
    out_h = nc.dram_tensor("out", [NB * 126, W3], F32, kind="ExternalOutput")

    # per-(sc,jc) engine assignment: square path and combine engine
    SQ_ENG = ["act"] * 12
    CMB_ENG = ["dve"] * 12

    with tile.TileContext(nc) as tc:
        with (
            tc.tile_pool(name="const", bufs=1) as cpool,
            tc.tile_pool(name="scratch", bufs=2) as spool,
            tc.tile_pool(name="hps", bufs=4, space="PSUM") as hpool,
            tc.tile_pool(name="ops", bufs=2, space="PSUM") as opool,
            tc.tile_pool(name="osb", bufs=3) as obpool,
        ):
            # ACT table warm-up: trigger exp table load immediately
            warm = cpool.tile([1, 8], F32)
            nc.vector.memset(warm[:, :], 0.0)
            nc.scalar.activation(warm[:, :], warm[:, :], Act.Exp)

            # d1 replicated across partitions: one tile per super-chunk so
            # the first Square only waits for its own slice's DMAs
            scs = _sc_slices(IPAD)
            d1_rep = {}
            for si, (a, b) in enumerate(scs):
                tl = cpool.tile([128, b - a], F32, tag=f"d1rep{si}")
                d1_rep[a] = tl
                for ka in range(a, b, 256):
                    kb = min(b, ka + 256)
                    nc.sync.dma_start(out=tl[:, ka - a:kb - a],
                                      in_=d1_h[0, ka:kb].partition_broadcast(128))
            d2c = cpool.tile([128, NJ2], F32)
            nc.sync.dma_start(out=d2c[:, :], in_=d2_h[:, :])
            s2_sb = cpool.tile([128, NJ2, W3P], F32R)
            for q in range(NJ2):
                nc.sync.dma_start(out=s2_sb[:, q, :],
                                  in_=s2_h[:, q * W3P:(q + 1) * W3P])
            s1_sb = cpool.tile([128, NBC, 126], F32R)
            for k in range(8):
                a = NBC * k // 8
                b = NBC * (k + 1) // 8
                if a < b:
                    nc.sync.dma_start(out=s1_sb[:, a:b, :],
                                      in_=s1_h[:, a * 126:b * 126])

            gg = cpool.tile([128, NJ2, IPAD], F32R)
            h_sb = cpool.tile([128, NIC, W3P], F32R)

            cp_k = 0
            inst_ptr = 0
            blk_open = {}
            g = 0
            for (a, b) in scs:
                w = b - a
                for q in range(NJ2):
                    sq = spool.tile([128, 1536], F32, tag="sq")
                    ex = spool.tile([128, 1536], F32, tag="ex")
                    se = SQ_ENG[g % len(SQ_ENG)]
                    if se == "act":
                        nc.scalar.activation(sq[:, :w], d1_rep[a][:, :w], Act.Square,
                                             bias=d2c[:, q:q + 1], scale=-sqrtc)
                    else:
                        dp = spool.tile([128, 1536], F32, tag="dp")
                        nc.vector.tensor_scalar(dp[:, :w], d1_rep[a][:, :w],
                                                -sqrtc, d2c[:, q:q + 1],
                                                op0=Alu.mult, op1=Alu.add)
                        if se == "dve":
                            nc.vector.tensor_tensor(sq[:, :w], dp[:, :w],
                                                    dp[:, :w], op=Alu.mult)
                        else:
                            nc.gpsimd.tensor_tensor(sq[:, :w], dp[:, :w],
                                                    dp[:, :w], op=Alu.mult)
                    nc.scalar.activation(ex[:, :w], sq[:, :w], Act.Exp,
                                         bias=lnc, scale=-0.5)
                    ce = CMB_ENG[g % len(CMB_ENG)]
                    if ce == "dve":
                        nc.vector.scalar_tensor_tensor(
                            gg[:, q, a:b], sq[:, :w], 1.0, ex[:, :w],
                            op0=Alu.subtract, op1=Alu.mult)
                    else:
                        t1 = spool.tile([128, 1536], F32, tag="t1")
                        nc.gpsimd.tensor_scalar(t1[:, :w], sq[:, :w], -1.0, None,
                                                op0=Alu.add)
                        nc.gpsimd.tensor_tensor(gg[:, q, a:b], t1[:, :w],
                                                ex[:, :w], op=Alu.mult)
                    g += 1
                # stage A over the i-chunks of this super-chunk
                for t in range(a // 128, b // 128):
                    for (cc0, ccw) in col_chunks:
                        h_ps = hpool.tile([128, 512], F32, tag="hps")
                        for q in range(NJ2):
                            nc.tensor.matmul(
                                h_ps[:, :ccw],
                                gg[:, q, t * 128:(t + 1) * 128],
                                s2_sb[:, q, cc0:cc0 + ccw],
                                start=(q == 0), stop=(q == NJ2 - 1))
                        nc.vector.tensor_copy(h_sb[:, t, cc0:cc0 + ccw],
                                              h_ps[:, :ccw])
                        cp_k += 1
                # stage B for blocks whose chunks are all covered now
                done_t = b // 128
                while inst_ptr < NBC and insts[inst_ptr][1] < done_t:
                    blk, t = insts[inst_ptr]
                    if blk not in blk_open:
                        blk_open[blk] = []
                    blk_open[blk].append(inst_ptr)
                    inst_ptr += 1
                    last_of_blk = (inst_ptr == NBC or insts[inst_ptr][0] != blk)
                    if not last_of_blk:
                        continue
                    ilist = blk_open.pop(blk)
                    o_sb = obpool.tile([126, W3], F32, tag="osb")
                    for (cc0, ccw) in col_chunks:
                        vw = min(W3 - cc0, ccw) if cc0 < W3 else 0
                        o_ps = opool.tile([126, 512], F32, tag="ops")
                        for k, ii in enumerate(ilist):
                            _, tt_ = insts[ii]
                            nc.tensor.matmul(
                                o_ps[:, :ccw], s1_sb[:, ii, :],
                                h_sb[:, tt_, cc0:cc0 + ccw],
                                start=(k == 0), stop=(k == len(ilist) - 1))
                        if vw > 0:
                            nc.vector.tensor_copy(o_sb[:, cc0:cc0 + vw],
                                                  o_ps[:, :vw])
                        cp_k += 1
                    nc.sync.dma_start(out=out_h[blk * 126:blk * 126 + 63, :],
                                      in_=o_sb[:63, :])
                    nc.sync.dma_start(out=out_h[blk * 126 + 63:(blk + 1) * 126, :],
                                      in_=o_sb[63:, :])
    nc.compile()
    return nc


def kernel(**inputs):
    global LAST_RESULTS
    d1 = np.asarray(inputs["d1"], dtype=np.float32).reshape(-1)
    u1 = np.asarray(inputs["u1"], dtype=np.float32)
    d2 = np.asarray(inputs["d2"], dtype=np.float32).reshape(-1)
    u2 = np.asarray(inputs["u2"], dtype=np.float32)
    ls = float(np.asarray(inputs["lengthscale"]).reshape(-1)[0])
    i1 = np.asarray(inputs["i1"]).reshape(-1).astype(np.int64)
    i2 = np.asarray(inputs["i2"]).reshape(-1).astype(np.int64)
    na1 = int(np.asarray(inputs["natoms1"]))
    na2 = int(np.asarray(inputs["natoms2"]))
    n1 = d1.shape[0]
    n2 = d2.shape[0]

    c = 1.0 / (ls * ls)
    sqrtc = math.sqrt(c)
    lnc = math.log(c)

    # ---- axis 1: sort by i1, pack tight to a multiple of 128 ----
    o1 = np.argsort(i1, kind="stable")
    d1s, u1s, i1s = d1[o1], u1[o1], i1[o1]
    IPAD = max(1, (n1 + 127) // 128) * 128
    d1p = np.zeros(IPAD, np.float32)
    d1p[:n1] = d1s
    nb = (na1 + ABLK - 1) // ABLK
    bnd = np.searchsorted(i1s, np.arange(nb + 1) * ABLK)
    bnd[-1] = n1
    insts = []
    for blk in range(nb):
        st, en = int(bnd[blk]), int(bnd[blk + 1])
        if en <= st:
            continue
        for t in range(st // 128, (en - 1) // 128 + 1):
            insts.append((blk, t))
    # order instances by chunk then block so stage B can stream in chunk order
    insts.sort(key=lambda bt: (bt[1], bt[0]))
    # regroup per block for contiguous-psum accumulation: sort by (block, chunk)
    # but emission needs "all chunks of block <= done"; keep (block-major) order
    insts.sort(key=lambda bt: (bt[0], bt[1]))
    NBC = len(insts)
    s1t = np.zeros((128, NBC, 126), np.float32)
    for ii, (blk, t) in enumerate(insts):
        st, en = int(bnd[blk]), int(bnd[blk + 1])
        k0, k1 = max(st, t * 128), min(en, (t + 1) * 128)
        ks = np.arange(k0, k1)
        p = ks - t * 128
        loc = (i1s[k0:k1] - blk * ABLK).astype(np.int64)
        for a in range(3):
            s1t[p, ii, 3 * loc + a] = -u1s[k0:k1, a]  # negated: sign trick
    insts = tuple(insts)

    # ---- axis 2: sort by i2, shard uniformly across cores ----
    o2 = np.argsort(i2, kind="stable")
    d2s, u2s, i2s = d2[o2], u2[o2], i2[o2]
    npc = (n2 + NCORES - 1) // NCORES
    P2 = max(1, (npc + 127) // 128) * 128
    NJ2 = P2 // 128
    lo = np.zeros(NCORES, np.int64)
    width = np.ones(NCORES, np.int64)
    core_slices = []
    for cc in range(NCORES):
        st = cc * npc
        en = min(n2, st + npc)
        core_slices.append((st, en))
        if en > st:
            lo[cc] = i2s[st]
            width[cc] = i2s[en - 1] - i2s[st] + 1
    W = int(width.max()) if n2 else 1
    W3 = 3 * W

    key = (IPAD, NJ2, W3, insts, nb, sqrtc, lnc)
    nc = _PROGRAM_CACHE.get(key)
    if nc is None:
        nc = _build_program(IPAD, NJ2, W3, insts, nb, sqrtc, lnc)
        _PROGRAM_CACHE[key] = nc

    in_maps = []
    for cc in range(NCORES):
        st, en = core_slices[cc]
        cnt = en - st
        d2col = np.zeros((NJ2, 128), np.float32)
        d2col.reshape(-1)[:cnt] = sqrtc * d2s[st:en]
        W3P = ((W3 + 255) // 256) * 256
        s2 = np.zeros((P2, W3P), np.float32)
        if cnt:
            rows = np.arange(cnt)
            loc = (i2s[st:en] - lo[cc]).astype(np.int64)
            for b in range(3):
                s2[rows, 3 * loc + b] = u2s[st:en, b]
        # partition-major layouts: [128, NJ2*W3P] and [128, NBC*126]
        s2_pm = np.ascontiguousarray(
            s2.reshape(NJ2, 128, W3P).transpose(1, 0, 2)).reshape(128, NJ2 * W3P)
        in_maps.append({
            "d1p": d1p.reshape(1, IPAD),
            "d2col": np.ascontiguousarray(d2col.T),
            "s2": s2_pm,
            "s1t": s1t.reshape(128, NBC * 126),
        })

    from concourse.bass_utils import run_bass_kernel_spmd
    if TRACE:
        _install_ntff_hook()
    res = run_bass_kernel_spmd(nc, in_maps, core_ids=list(range(NCORES)),
                               trace=TRACE)
    LAST_RESULTS = res

    out = np.zeros((3 * na1, 3 * na2), np.float32)
    for cc in range(NCORES):
        st, en = core_slices[cc]
        if en <= st:
            continue
        w3 = 3 * int(width[cc])
        col0 = 3 * int(lo[cc])
        out[:, col0:col0 + w3] += res.results[cc]["out"][:3 * na1, :w3]
    return out


# revision 15
# speedup vs baseline: 1.2405x; 1.2405x over previous
"""DSimilarity.gradgrad force-force covariance block on 8 Trainium2 cores.

out[m*3+a, n*3+b] = sum_{i,j} u1[i,a]*u2[j,b]*gg[i,j]*[i1[i]==m]*[i2[j]==n]
with gg[i,j] = (c - c^2 diff^2) * exp(-0.5 c diff^2), diff = d1[i]-d2[j], c=1/l^2.

Strategy: out = S1T.T @ gg @ S2 with sparse scatter matrices densified after
sorting pairs by atom index. Axis-2 (j) is sorted by i2 and sharded 1/8 per
core -> each core produces a contiguous strip of output columns (overlap-add
at boundary atoms on the host). Axis-1 (i) is sorted by i1, packed tight to a
multiple of 128; stage B runs per 42-atom row block over the i-chunks that
block's pairs touch (boundary chunks appear in two blocks with disjoint
nonzero rows). gg never touches HBM: ACT/DVE/GpSimd compute it in SBUF
super-chunks, PE consumes it as matmul weights. d1 is replicated across
partitions with a K=1 ones-matmul (also warms the PE during the DMA preamble).
"""

import math
import sys
import types

import numpy as np

NCORES = 8
ABLK = 42  # atoms per stage-B row block (126 rows)

TRACE = False  # test.py sets True to capture an NTFF profile
LAST_RESULTS = None  # BassKernelResults of the last run (for test.py)

_PROGRAM_CACHE = {}


def _install_ntff_hook():
    try:
        from antenv.axon_hooks import get_axon_ntff_profile_hook  # noqa: F401
        return
    except ImportError:
        pass
    try:
        from trn_agent_boot.trn_boot import _ntff_profile_via_ctypes
        import antenv
        hook = _ntff_profile_via_ctypes('/opt/axon/libaxon_pjrt.so')
        mod = types.ModuleType("antenv.axon_hooks")
        mod._hook = hook
        mod.get_axon_ntff_profile_hook = lambda: mod._hook
        mod.set_axon_ntff_profile_hook = lambda h: setattr(mod, "_hook", h)
        antenv.axon_hooks = mod
        sys.modules["antenv.axon_hooks"] = mod
    except Exception:
        pass


def _sc_slices(ipad):
    """Split [0, ipad) into super-chunks of ~1536 (multiples of 128)."""
    out = []
    a = 0
    while a < ipad:
        w = min(1536, ipad - a)
        out.append((a, a + w))
        a += w
    return out


def _build_program(IPAD, NJ2, W3, insts, NBTOT, sqrtc, lnc):
    """Compile the per-core Bass program (same program on all 8 cores).

    insts: tuple of (block, chunk) stage-B instances, block-major.
    Matmuls run in float32r (tf32-class) with the moving dim padded to >=256
    so the PE streams 1 column/cycle; elementwise gg stays exact fp32.
    """
    import concourse.bacc as bacc
    import concourse.tile as tile
    import concourse.mybir as mybir

    F32 = mybir.dt.float32
    F32R = mybir.dt.float32r
    Alu = mybir.AluOpType
    Act = mybir.ActivationFunctionType

    NIC = IPAD // 128
    NBC = len(insts)
    NB = NBTOT

    # pad the stage-A/B moving dim to a multiple of 256, chunks of <=512
    W3P = ((W3 + 255) // 256) * 256
    col_chunks = []
    c0 = 0
    while c0 < W3P:
        col_chunks.append((c0, min(512, W3P - c0)))
        c0 += 512

    nc = bacc.Bacc("TRN2", target_bir_lowering=False, debug=False)
    d1_h = nc.dram_tensor("d1p", [1, IPAD], F32, kind="ExternalInput")
    d2_h = nc.dram_tensor("d2col", [128, NJ2], F32, kind="ExternalInput")
    s2_h = nc.dram_tensor("s2", [128, NJ2 * W3P], F32R, kind="ExternalInput")
    s1_h = nc.dram_tensor("s1t", [128, NBC * 126], F32R, kind="ExternalInput")
    out_h = nc.dram_tensor("out", [NB * 126, W3], F32, kind="ExternalOutput")

    # per-(sc,jc) engine assignment: square path and combine engine
    SQ_ENG = ["act", "act", "dve", "act", "act", "dve", "act", "act",
              "dve", "act", "act", "act"]
    CMB_ENG = ["dve"] * 12

    with tile.TileContext(nc) as tc:
        with (
            tc.tile_pool(name="const", bufs=1) as cpool,
            tc.tile_pool(name="scratch", bufs=2) as spool,
            tc.tile_pool(name="hps", bufs=4, space="PSUM") as hpool,
            tc.tile_pool(name="ops", bufs=2, space="PSUM") as opool,
            tc.tile_pool(name="osb", bufs=3) as obpool,
        ):
            # ACT table warm-up: trigger exp table load immediately
            warm = cpool.tile([1, 8], F32)
            nc.vector.memset(warm[:, :], 0.0)
            nc.scalar.activation(warm[:, :], warm[:, :], Act.Exp)

            # input DMAs: d2 first (gates the first Square), then per-sc
            # d1 broadcasts, then s2, then s1t. One dma_start each: a single
            # InstDMACopy already spreads across all 16 SDMA engines, and
            # fewer descriptors keeps the SP HWDGE FIFO short.
            d2c = cpool.tile([128, NJ2], F32)
            nc.sync.dma_start(out=d2c[:, :], in_=d2_h[:, :])
            scs = _sc_slices(IPAD)
            d1_rep = {}
            for si, (a, b) in enumerate(scs):
                tl = cpool.tile([128, b - a], F32, tag=f"d1rep{si}")
                d1_rep[a] = tl
                nc.sync.dma_start(out=tl[:, :],
                                  in_=d1_h[0, a:b].partition_broadcast(128))
            s2_sb = cpool.tile([128, NJ2, W3P], F32R)
            nc.sync.dma_start(out=s2_sb[:, :, :],
                              in_=s2_h[:, :].rearrange("p (q w) -> p q w", q=NJ2))
            s1_sb = cpool.tile([128, NBC, 126], F32R)
            nc.sync.dma_start(out=s1_sb[:, :, :],
                              in_=s1_h[:, :].rearrange("p (i m) -> p i m", i=NBC))

            gg = cpool.tile([128, NJ2, IPAD], F32R)
            h_sb = cpool.tile([128, NIC, W3P], F32R)

            cp_k = 0
            inst_ptr = 0
            blk_open = {}
            g = 0
            for (a, b) in scs:
                w = b - a
                for q in range(NJ2):
                    sq = spool.tile([128, 1536], F32, tag="sq")
                    ex = spool.tile([128, 1536], F32, tag="ex")
                    se = SQ_ENG[g % len(SQ_ENG)]
                    if se == "act":
                        nc.scalar.activation(sq[:, :w], d1_rep[a][:, :w], Act.Square,
                                             bias=d2c[:, q:q + 1], scale=-sqrtc)
                    else:
                        dp = spool.tile([128, 1536], F32, tag="dp")
                        nc.vector.tensor_scalar(dp[:, :w], d1_rep[a][:, :w],
                                                -sqrtc, d2c[:, q:q + 1],
                                                op0=Alu.mult, op1=Alu.add)
                        if se == "dve":
                            nc.vector.tensor_tensor(sq[:, :w], dp[:, :w],
                                                    dp[:, :w], op=Alu.mult)
                        else:
                            nc.gpsimd.tensor_tensor(sq[:, :w], dp[:, :w],
                                                    dp[:, :w], op=Alu.mult)
                    nc.scalar.activation(ex[:, :w], sq[:, :w], Act.Exp,
                                         bias=lnc, scale=-0.5)
                    ce = CMB_ENG[g % len(CMB_ENG)]
                    if ce == "dve":
                        nc.vector.scalar_tensor_tensor(
                            gg[:, q, a:b], sq[:, :w], 1.0, ex[:, :w],
                            op0=Alu.subtract, op1=Alu.mult)
                    else:
                        t1 = spool.tile([128, 1536], F32, tag="t1")
                        nc.gpsimd.tensor_scalar(t1[:, :w], sq[:, :w], -1.0, None,
                                                op0=Alu.add)
                        nc.gpsimd.tensor_tensor(gg[:, q, a:b], t1[:, :w],
                                                ex[:, :w], op=Alu.mult)
                    g += 1
                # stage A over the i-chunks of this super-chunk
                for t in range(a // 128, b // 128):
                    for (cc0, ccw) in col_chunks:
                        h_ps = hpool.tile([128, 512], F32, tag="hps")
                        for q in range(NJ2):
                            nc.tensor.matmul(
                                h_ps[:, :ccw],
                                gg[:, q, t * 128:(t + 1) * 128],
                                s2_sb[:, q, cc0:cc0 + ccw],
                                start=(q == 0), stop=(q == NJ2 - 1))
                        if cp_k % 3 == 2:
                            nc.scalar.copy(h_sb[:, t, cc0:cc0 + ccw],
                                           h_ps[:, :ccw])
                        else:
                            nc.vector.tensor_copy(h_sb[:, t, cc0:cc0 + ccw],
                                                  h_ps[:, :ccw])
                        cp_k += 1
                # stage B for blocks whose chunks are all covered now
                done_t = b // 128
                while inst_ptr < NBC and insts[inst_ptr][1] < done_t:
                    blk, t = insts[inst_ptr]
                    if blk not in blk_open:
                        blk_open[blk] = []
                    blk_open[blk].append(inst_ptr)
                    inst_ptr += 1
                    last_of_blk = (inst_ptr == NBC or insts[inst_ptr][0] != blk)
                    if not last_of_blk:
                        continue
                    ilist = blk_open.pop(blk)
                    o_sb = obpool.tile([126, W3], F32, tag="osb")
                    for (cc0, ccw) in col_chunks:
                        vw = min(W3 - cc0, ccw) if cc0 < W3 else 0
                        o_ps = opool.tile([126, 512], F32, tag="ops")
                        for k, ii in enumerate(ilist):
                            _, tt_ = insts[ii]
                            nc.tensor.matmul(
                                o_ps[:, :ccw], s1_sb[:, ii, :],
                                h_sb[:, tt_, cc0:cc0 + ccw],
                                start=(k == 0), stop=(k == len(ilist) - 1))
                        if vw > 0:
                            nc.vector.tensor_copy(o_sb[:, cc0:cc0 + vw],
                                                  o_ps[:, :vw])
                        cp_k += 1
                    nc.scalar.dma_start(out=out_h[blk * 126:(blk + 1) * 126, :],
                                        in_=o_sb[:, :])
    nc.compile()
    return nc


def kernel(**inputs):
    global LAST_RESULTS
    d1 = np.asarray(inputs["d1"], dtype=np.float32).reshape(-1)
    u1 = np.asarray(inputs["u1"], dtype=np.float32)
    d2 = np.asarray(inputs["d2"], dtype=np.float32).reshape(-1)
    u2 = np.asarray(inputs["u2"], dtype=np.float32)
    ls = float(np.asarray(inputs["lengthscale"]).reshape(-1)[0])
    i1 = np.asarray(inputs["i1"]).reshape(-1).astype(np.int64)
    i2 = np.asarray(inputs["i2"]).reshape(-1).astype(np.int64)
    na1 = int(np.asarray(inputs["natoms1"]))
    na2 = int(np.asarray(inputs["natoms2"]))
    n1 = d1.shape[0]
    n2 = d2.shape[0]

    c = 1.0 / (ls * ls)
    sqrtc = math.sqrt(c)
    lnc = math.log(c)

    # ---- axis 1: sort by i1, pack tight to a multiple of 128 ----
    o1 = np.argsort(i1, kind="stable")
    d1s, u1s, i1s = d1[o1], u1[o1], i1[o1]
    IPAD = max(1, (n1 + 127) // 128) * 128
    d1p = np.zeros(IPAD, np.float32)
    d1p[:n1] = d1s
    nb = (na1 + ABLK - 1) // ABLK
    bnd = np.searchsorted(i1s, np.arange(nb + 1) * ABLK)
    bnd[-1] = n1
    insts = []
    for blk in range(nb):
        st, en = int(bnd[blk]), int(bnd[blk + 1])
        if en <= st:
            continue
        for t in range(st // 128, (en - 1) // 128 + 1):
            insts.append((blk, t))
    # order instances by chunk then block so stage B can stream in chunk order
    insts.sort(key=lambda bt: (bt[1], bt[0]))
    # regroup per block for contiguous-psum accumulation: sort by (block, chunk)
    # but emission needs "all chunks of block <= done"; keep (block-major) order
    insts.sort(key=lambda bt: (bt[0], bt[1]))
    NBC = len(insts)
    s1t = np.zeros((128, NBC, 126), np.float32)
    for ii, (blk, t) in enumerate(insts):
        st, en = int(bnd[blk]), int(bnd[blk + 1])
        k0, k1 = max(st, t * 128), min(en, (t + 1) * 128)
        ks = np.arange(k0, k1)
        p = ks - t * 128
        loc = (i1s[k0:k1] - blk * ABLK).astype(np.int64)
        for a in range(3):
            s1t[p, ii, 3 * loc + a] = -u1s[k0:k1, a]  # negated: sign trick
    insts = tuple(insts)

    # ---- axis 2: sort by i2, shard uniformly across cores ----
    o2 = np.argsort(i2, kind="stable")
    d2s, u2s, i2s = d2[o2], u2[o2], i2[o2]
    npc = (n2 + NCORES - 1) // NCORES
    P2 = max(1, (npc + 127) // 128) * 128
    NJ2 = P2 // 128
    lo = np.zeros(NCORES, np.int64)
    width = np.ones(NCORES, np.int64)
    core_slices = []
    for cc in range(NCORES):
        st = cc * npc
        en = min(n2, st + npc)
        core_slices.append((st, en))
        if en > st:
            lo[cc] = i2s[st]
            width[cc] = i2s[en - 1] - i2s[st] + 1
    W = int(width.max()) if n2 else 1
    W3 = 3 * W

    key = (IPAD, NJ2, W3, insts, nb, sqrtc, lnc)
    nc = _PROGRAM_CACHE.get(key)
    if nc is None:
        nc = _build_program(IPAD, NJ2, W3, insts, nb, sqrtc, lnc)
        _PROGRAM_CACHE[key] = nc

    in_maps = []
    for cc in range(NCORES):
        st, en = core_slices[cc]
        cnt = en - st
        d2col = np.zeros((NJ2, 128), np.float32)
        d2col.reshape(-1)[:cnt] = sqrtc * d2s[st:en]
        W3P = ((W3 + 255) // 256) * 256
        s2 = np.zeros((P2, W3P), np.float32)
        if cnt:
            rows = np.arange(cnt)
            loc = (i2s[st:en] - lo[cc]).astype(np.int64)
            for b in range(3):
                s2[rows, 3 * loc + b] = u2s[st:en, b]
        # partition-major layouts: [128, NJ2*W3P] and [128, NBC*126]
        s2_pm = np.ascontiguousarray(
            s2.reshape(NJ2, 128, W3P).transpose(1, 0, 2)).reshape(128, NJ2 * W3P)
        in_maps.append({
            "d1p": d1p.reshape(1, IPAD),
            "d2col": np.ascontiguousarray(d2col.T),
            "s2": s2_pm,
            "s1t": s1t.reshape(128, NBC * 126),
        })

    from concourse.bass_utils import run_bass_kernel_spmd
    if TRACE:
        _install_ntff_hook()
    res = run_bass_kernel_spmd(nc, in_maps, core_ids=list(range(NCORES)),
                               trace=TRACE)
    LAST_RESULTS = res

    out = np.zeros((3 * na1, 3 * na2), np.float32)
    for cc in range(NCORES):
        st, en = core_slices[cc]
        if en <= st:
            continue
        w3 = 3 * int(width[cc])
        col0 = 3 * int(lo[cc])
        out[:, col0:col0 + w3] += res.results[cc]["out"][:3 * na1, :w3]
    return out


# revision 19
# speedup vs baseline: 1.2426x; 1.0016x over previous
"""DSimilarity.gradgrad force-force covariance block on 8 Trainium2 cores.

out[m*3+a, n*3+b] = sum_{i,j} u1[i,a]*u2[j,b]*gg[i,j]*[i1[i]==m]*[i2[j]==n]
with gg[i,j] = (c - c^2 diff^2) * exp(-0.5 c diff^2), diff = d1[i]-d2[j], c=1/l^2.

Strategy: out = S1T.T @ gg @ S2 with sparse scatter matrices densified after
sorting pairs by atom index. Axis-2 (j) is sorted by i2 and sharded 1/8 per
core -> each core produces a contiguous strip of output columns (overlap-add
at boundary atoms on the host). Axis-1 (i) is sorted by i1, packed tight to a
multiple of 128; stage B runs per 42-atom row block over the i-chunks that
block's pairs touch (boundary chunks appear in two blocks with disjoint
nonzero rows). gg never touches HBM: ACT/DVE/GpSimd compute it in SBUF
super-chunks, PE consumes it as matmul weights. d1 is replicated across
partitions with a K=1 ones-matmul (also warms the PE during the DMA preamble).
"""

import math
import sys
import types

import numpy as np

NCORES = 8
ABLK = 42  # atoms per stage-B row block (126 rows)

TRACE = False  # test.py sets True to capture an NTFF profile
LAST_RESULTS = None  # BassKernelResults of the last run (for test.py)

_PROGRAM_CACHE = {}


def _install_ntff_hook():
    try:
        from antenv.axon_hooks import get_axon_ntff_profile_hook  # noqa: F401
        return
    except ImportError:
        pass
    try:
        from trn_agent_boot.trn_boot import _ntff_profile_via_ctypes
        import antenv
        hook = _ntff_profile_via_ctypes('/opt/axon/libaxon_pjrt.so')
        mod = types.ModuleType("antenv.axon_hooks")
        mod._hook = hook
        mod.get_axon_ntff_profile_hook = lambda: mod._hook
        mod.set_axon_ntff_profile_hook = lambda h: setattr(mod, "_hook", h)
        antenv.axon_hooks = mod
        sys.modules["antenv.axon_hooks"] = mod
    except Exception:
        pass


def _sc_slices(ipad):
    """Split [0, ipad): small first slice (fast start), ~1664 after."""
    out = []
    a = 0
    first = True
    while a < ipad:
        w = min(768 if first else 1664, ipad - a)
        out.append((a, a + w))
        a += w
        first = False
    return out


def _build_program(IPAD, NJ2, W3, insts, NBTOT, sqrtc, lnc):
    """Compile the per-core Bass program (same program on all 8 cores).

    insts: tuple of (block, chunk) stage-B instances, block-major.
    Matmuls run in float32r (tf32-class) with the moving dim padded to >=256
    so the PE streams 1 column/cycle; elementwise gg stays exact fp32.
    """
    import concourse.bacc as bacc
    import concourse.tile as tile
    import concourse.mybir as mybir

    F32 = mybir.dt.float32
    F32R = mybir.dt.float32r
    Alu = mybir.AluOpType
    Act = mybir.ActivationFunctionType

    NIC = IPAD // 128
    NBC = len(insts)
    NB = NBTOT

    # pad the stage-A/B moving dim to a multiple of 256, chunks of <=512
    W3P = ((W3 + 255) // 256) * 256
    col_chunks = []
    c0 = 0
    while c0 < W3P:
        col_chunks.append((c0, min(512, W3P - c0)))
        c0 += 512

    nc = bacc.Bacc("TRN2", target_bir_lowering=False, debug=False)
    d1_h = nc.dram_tensor("d1p", [1, IPAD], F32, kind="ExternalInput")
    d2_h = nc.dram_tensor("d2col", [128, NJ2], F32, kind="ExternalInput")
    s2_h = nc.dram_tensor("s2", [128, NJ2 * W3P], F32R, kind="ExternalInput")
    s1_h = nc.dram_tensor("s1t", [128, NBC * 126], F32R, kind="ExternalInput")
    out_h = nc.dram_tensor("out", [NB * 126, W3], F32, kind="ExternalOutput")

    with tile.TileContext(nc) as tc:
        with (
            tc.tile_pool(name="const", bufs=1) as cpool,
            tc.tile_pool(name="scratch", bufs=3) as spool,
            tc.tile_pool(name="hps", bufs=4, space="PSUM") as hpool,
            tc.tile_pool(name="ops", bufs=2, space="PSUM") as opool,
            tc.tile_pool(name="osb", bufs=3) as obpool,
        ):
            # ACT table warm-up: trigger exp table load immediately
            warm = cpool.tile([1, 8], F32)
            nc.vector.memset(warm[:, :], 0.0)
            nc.scalar.activation(warm[:, :], warm[:, :], Act.Exp)

            # input DMAs: d2 first (gates the first Square), then per-sc
            # d1 broadcasts, then s2, then s1t. One dma_start each: a single
            # InstDMACopy already spreads across all 16 SDMA engines, and
            # fewer descriptors keeps the SP HWDGE FIFO short.
            d2c = cpool.tile([128, NJ2], F32)
            nc.sync.dma_start(out=d2c[:, :], in_=d2_h[:, :])
            scs = _sc_slices(IPAD)
            d1_rep = {}
            for si, (a, b) in enumerate(scs):
                tl = cpool.tile([128, b - a], F32, tag=f"d1rep{si}")
                d1_rep[a] = tl
                nc.sync.dma_start(out=tl[:, :],
                                  in_=d1_h[0, a:b].partition_broadcast(128))
            s2_sb = cpool.tile([128, NJ2, W3P], F32R)
            nc.sync.dma_start(out=s2_sb[:, :, :],
                              in_=s2_h[:, :].rearrange("p (q w) -> p q w", q=NJ2))
            s1_sb = cpool.tile([128, NBC, 126], F32R)
            nc.sync.dma_start(out=s1_sb[:, :, :],
                              in_=s1_h[:, :].rearrange("p (i m) -> p i m", i=NBC))

            scw_max = max(b - a for a, b in scs)
            with tc.tile_pool(name="ggp", bufs=2) as ggpool:
                _run_body(nc, tc, tile, mybir, cpool, spool, hpool, opool,
                          obpool, ggpool, scs, d1_rep, d2c, s2_sb, s1_sb,
                          out_h, insts, col_chunks, W3, W3P, NJ2, NBC,
                          sqrtc, lnc, scw_max)
    nc.compile()
    return nc


def _run_body(nc, tc, tile, mybir, cpool, spool, hpool, opool, obpool,
              ggpool, scs, d1_rep, d2c, s2_sb, s1_sb, out_h, insts,
              col_chunks, W3, W3P, NJ2, NBC, sqrtc, lnc, scw_max):
    F32 = mybir.dt.float32
    F32R = mybir.dt.float32r
    Alu = mybir.AluOpType
    Act = mybir.ActivationFunctionType
    SQ_ENG = ["act", "act", "dve", "act", "act", "dve", "act", "act",
              "dve", "act", "act", "act"]
    CMB_ENG = ["dve"] * 12
    # chunk index -> (sc index, local chunk offset)
    t2sc = {}
    h_tiles = []
    for si, (a, b) in enumerate(scs):
        nch = (b - a) // 128
        h_tiles.append(cpool.tile([128, nch, W3P], F32R, tag=f"h{si}", name=f"h{si}"))
        for tl in range(nch):
            t2sc[a // 128 + tl] = (si, tl)
    if True:
            cp_k = 0
            inst_ptr = 0
            blk_open = {}
            g = 0
            for si, (a, b) in enumerate(scs):
                w = b - a
                gg = ggpool.tile([128, NJ2, scw_max], F32R, tag="gg")
                for q in range(NJ2):
                    sq = spool.tile([128, scw_max], F32, tag="sq")
                    ex = spool.tile([128, scw_max], F32, tag="ex")
                    se = SQ_ENG[g % len(SQ_ENG)]
                    if se == "act":
                        nc.scalar.activation(sq[:, :w], d1_rep[a][:, :w], Act.Square,
                                             bias=d2c[:, q:q + 1], scale=-sqrtc)
                    else:
                        dp = spool.tile([128, scw_max], F32, tag="dp")
                        nc.vector.tensor_scalar(dp[:, :w], d1_rep[a][:, :w],
                                                -sqrtc, d2c[:, q:q + 1],
                                                op0=Alu.mult, op1=Alu.add)
                        if se == "dve":
                            nc.vector.tensor_tensor(sq[:, :w], dp[:, :w],
                                                    dp[:, :w], op=Alu.mult)
                        else:
                            nc.gpsimd.tensor_tensor(sq[:, :w], dp[:, :w],
                                                    dp[:, :w], op=Alu.mult)
                    nc.scalar.activation(ex[:, :w], sq[:, :w], Act.Exp,
                                         bias=lnc, scale=-0.5)
                    ce = CMB_ENG[g % len(CMB_ENG)]
                    if ce == "dve":
                        nc.vector.scalar_tensor_tensor(
                            gg[:, q, 0:w], sq[:, :w], 1.0, ex[:, :w],
                            op0=Alu.subtract, op1=Alu.mult)
                    else:
                        t1 = spool.tile([128, scw_max], F32, tag="t1")
                        nc.gpsimd.tensor_scalar(t1[:, :w], sq[:, :w], -1.0, None,
                                                op0=Alu.add)
                        nc.gpsimd.tensor_tensor(gg[:, q, 0:w], t1[:, :w],
                                                ex[:, :w], op=Alu.mult)
                    g += 1
                # stage A over the i-chunks of this super-chunk
                for t in range(a // 128, b // 128):
                    tl = t - a // 128
                    for (cc0, ccw) in col_chunks:
                        h_ps = hpool.tile([128, 512], F32, tag="hps")
                        for q in range(NJ2):
                            nc.tensor.matmul(
                                h_ps[:, :ccw],
                                gg[:, q, tl * 128:(tl + 1) * 128],
                                s2_sb[:, q, cc0:cc0 + ccw],
                                start=(q == 0), stop=(q == NJ2 - 1))
                        if cp_k % 3 == 2:
                            nc.scalar.copy(h_tiles[si][:, tl, cc0:cc0 + ccw],
                                           h_ps[:, :ccw])
                        else:
                            nc.vector.tensor_copy(
                                h_tiles[si][:, tl, cc0:cc0 + ccw],
                                h_ps[:, :ccw])
                        cp_k += 1
                # stage B for blocks whose chunks are all covered now
                done_t = b // 128
                while inst_ptr < NBC and insts[inst_ptr][1] < done_t:
                    blk, t = insts[inst_ptr]
                    if blk not in blk_open:
                        blk_open[blk] = []
                    blk_open[blk].append(inst_ptr)
                    inst_ptr += 1
                    last_of_blk = (inst_ptr == NBC or insts[inst_ptr][0] != blk)
                    if not last_of_blk:
                        continue
                    ilist = blk_open.pop(blk)
                    o_sb = obpool.tile([126, W3], F32, tag="osb")
                    for (cc0, ccw) in col_chunks:
                        vw = min(W3 - cc0, ccw) if cc0 < W3 else 0
                        o_ps = opool.tile([126, 512], F32, tag="ops")
                        for k, ii in enumerate(ilist):
                            _, tt_ = insts[ii]
                            tsi, tloc = t2sc[tt_]
                            nc.tensor.matmul(
                                o_ps[:, :ccw], s1_sb[:, ii, :],
                                h_tiles[tsi][:, tloc, cc0:cc0 + ccw],
                                start=(k == 0), stop=(k == len(ilist) - 1))
                        if vw > 0:
                            nc.vector.tensor_copy(o_sb[:, cc0:cc0 + vw],
                                                  o_ps[:, :vw])
                        cp_k += 1
                    nc.scalar.dma_start(out=out_h[blk * 126:(blk + 1) * 126, :],
                                        in_=o_sb[:, :])
    nc.compile()
    return nc


def kernel(**inputs):
    global LAST_RESULTS
    d1 = np.asarray(inputs["d1"], dtype=np.float32).reshape(-1)
    u1 = np.asarray(inputs["u1"], dtype=np.float32)
    d2 = np.asarray(inputs["d2"], dtype=np.float32).reshape(-1)
    u2 = np.asarray(inputs["u2"], dtype=np.float32)
    ls = float(np.asarray(inputs["lengthscale"]).reshape(-1)[0])
    i1 = np.asarray(inputs["i1"]).reshape(-1).astype(np.int64)
    i2 = np.asarray(inputs["i2"]).reshape(-1).astype(np.int64)
    na1 = int(np.asarray(inputs["natoms1"]))
    na2 = int(np.asarray(inputs["natoms2"]))
    n1 = d1.shape[0]
    n2 = d2.shape[0]

    c = 1.0 / (ls * ls)
    sqrtc = math.sqrt(c)
    lnc = math.log(c)

    # ---- axis 1: sort by i1, pack tight to a multiple of 128 ----
    o1 = np.argsort(i1, kind="stable")
    d1s, u1s, i1s = d1[o1], u1[o1], i1[o1]
    IPAD = max(1, (n1 + 127) // 128) * 128
    d1p = np.zeros(IPAD, np.float32)
    d1p[:n1] = d1s
    nb = (na1 + ABLK - 1) // ABLK
    bnd = np.searchsorted(i1s, np.arange(nb + 1) * ABLK)
    bnd[-1] = n1
    insts = []
    for blk in range(nb):
        st, en = int(bnd[blk]), int(bnd[blk + 1])
        if en <= st:
            continue
        for t in range(st // 128, (en - 1) // 128 + 1):
            insts.append((blk, t))
    # order instances by chunk then block so stage B can stream in chunk order
    insts.sort(key=lambda bt: (bt[1], bt[0]))
    # regroup per block for contiguous-psum accumulation: sort by (block, chunk)
    # but emission needs "all chunks of block <= done"; keep (block-major) order
    insts.sort(key=lambda bt: (bt[0], bt[1]))
    NBC = len(insts)
    s1t = np.zeros((128, NBC, 126), np.float32)
    for ii, (blk, t) in enumerate(insts):
        st, en = int(bnd[blk]), int(bnd[blk + 1])
        k0, k1 = max(st, t * 128), min(en, (t + 1) * 128)
        ks = np.arange(k0, k1)
        p = ks - t * 128
        loc = (i1s[k0:k1] - blk * ABLK).astype(np.int64)
        for a in range(3):
            s1t[p, ii, 3 * loc + a] = -u1s[k0:k1, a]  # negated: sign trick
    insts = tuple(insts)

    # ---- axis 2: sort by i2, shard uniformly across cores ----
    o2 = np.argsort(i2, kind="stable")
    d2s, u2s, i2s = d2[o2], u2[o2], i2[o2]
    npc = (n2 + NCORES - 1) // NCORES
    P2 = max(1, (npc + 127) // 128) * 128
    NJ2 = P2 // 128
    lo = np.zeros(NCORES, np.int64)
    width = np.ones(NCORES, np.int64)
    core_slices = []
    for cc in range(NCORES):
        st = cc * npc
        en = min(n2, st + npc)
        core_slices.append((st, en))
        if en > st:
            lo[cc] = i2s[st]
            width[cc] = i2s[en - 1] - i2s[st] + 1
    W = int(width.max()) if n2 else 1
    W3 = 3 * W

    key = (IPAD, NJ2, W3, insts, nb, sqrtc, lnc)
    nc = _PROGRAM_CACHE.get(key)
    if nc is None:
        nc = _build_program(IPAD, NJ2, W3, insts, nb, sqrtc, lnc)
        _PROGRAM_CACHE[key] = nc

    in_maps = []
    for cc in range(NCORES):
        st, en = core_slices[cc]
        cnt = en - st
        d2col = np.zeros((NJ2, 128), np.float32)
        d2col.reshape(-1)[:cnt] = sqrtc * d2s[st:en]
        W3P = ((W3 + 255) // 256) * 256
        s2 = np.zeros((P2, W3P), np.float32)
        if cnt:
            rows = np.arange(cnt)
            loc = (i2s[st:en] - lo[cc]).astype(np.int64)
            for b in range(3):
                s2[rows, 3 * loc + b] = u2s[st:en, b]
        # partition-major layouts: [128, NJ2*W3P] and [128, NBC*126]
        s2_pm = np.ascontiguousarray(
            s2.reshape(NJ2, 128, W3P).transpose(1, 0, 2)).reshape(128, NJ2 * W3P)
        in_maps.append({
            "d1p": d1p.reshape(1, IPAD),
            "d2col": np.ascontiguousarray(d2col.T),
            "s2": s2_pm,
            "s1t": s1t.reshape(128, NBC * 126),
        })

    from concourse.bass_utils import run_bass_kernel_spmd
    if TRACE:
        _install_ntff_hook()
    res = run_bass_kernel_spmd(nc, in_maps, core_ids=list(range(NCORES)),
                               trace=TRACE)
    LAST_RESULTS = res

    out = np.zeros((3 * na1, 3 * na2), np.float32)
    for cc in range(NCORES):
        st, en = core_slices[cc]
        if en <= st:
            continue
        w3 = 3 * int(width[cc])
        col0 = 3 * int(lo[cc])
        out[:, col0:col0 + w3] += res.results[cc]["out"][:3 * na1, :w3]
    return out
